# revision 27
# baseline (speedup 1.0000x reference)
"""
Trainium2 Bass kernel for nn_Attention_335007449901 (sparse window attention).

Model (per image, eval mode):
  q = BN(conv1x1(x, wq)); k = BN(conv1x1(x, wk)); v = BN(conv1x1(x, wv))
  7x7 windows over the 112x112 image -> T=256 window tokens, token
  features = (channel, within-window position p) pairs.
  dots[i,j] = <q_i, k_j> * 0.125 ; attn = softmax_j ; out = attn @ v
  y = gelu(out); z = BN(conv1x1(y, wo) + bo); out = gelu(z + x)

Sharding: pure data parallel over batch, 4 images per core on 8 cores.

Scheme (v3):
  * Window permute on the host both ways; device sees win layout
    [c, p*T + j] contiguous. BNs folded into weights on the host;
    q/k never materialize (dots_T via u = M^T x); k bias drops; the
    q-bias row c[j] via 4-up col-tiled M=1 matmuls.
  * Contraction reorder for the value path: av = wv @ (x @ attn)
    instead of (wv x) @ attn -- the attention-average contracts over
    the 128 input channels, not 256 hidden ones. xa = x @ attn runs
    as one fp8 DoubleRow matmul per position (contraction 256 = both
    token halves, stationary = a host-provided token-major fp8 copy
    of x), then av = wv @ xa in bf16 with a constant stationary wv.
    This kills the per-position v-conv casts entirely.
  * The out-conv runs in fp8 DoubleRow (contraction 256 = both hidden
    halves) with stationary 16*wo. The residual rides the same PSUM
    group as a 16*I identity matmul (emitted first, start=True), so
    gelu2 reads PSUM directly with scale=1/16.
  * Scales keep fp8 in range: xT8 = 8*x, attn8 = 8*attn (via 0.125
    ones in the softmax-sum matmul), wo8 = 16*wo; the 1/64 and 1/16
    fold into the gelus' input scale; Bv/Bo ride the gelus'
    per-partition bias (Bv passes through because attn rows sum to 1).
  * Phase 2 works in supergroups of 4 positions: av PSUM is one
    [128, 1024] 2-bank tile per (SG, kc) so gelu1 is a single ACT
    instruction with a uniform per-partition bias.
  * Cross-image software pipeline: phase 1 of image i+1 (u-convs,
    dots, c-trick -- PE/DVE heavy, ACT idle) is emitted interleaved
    into phase 2 of image i (ACT heavy), two steps per supergroup,
    so no engine sits idle at image boundaries. xa matmuls run one
    SG ahead of their av consumers to hide the DVE cast latency.
  * Engine split: PE matmuls; ACT exp + both gelus; DVE all
    PSUM->SBUF casts + reciprocal; GPSIMD the softmax normalize muls
    (it cannot read PSUM).
"""

import numpy as np

IN_C = 128
HIDE_C = 256
HC2 = 128
OUT_C = 128
WS = 7
SCALE = 0.125
EPS = 1e-5
B, H, W = 32, 112, 112
HW = H * W          # 12544
H1 = H // WS        # 16
W1 = W // WS        # 16
T = H1 * W1         # 256 windows
NP = WS * WS        # 49 positions
NCORES = 8
BPC = B // NCORES   # images per core

F32 = np.float32


def build_bass_kernel(bpc=BPC):
    import concourse.bass as bass
    import concourse.tile as tile
    import concourse.mybir as mybir
    from concourse import bacc

    f32 = mybir.dt.float32
    bf16 = mybir.dt.bfloat16
    fp8 = mybir.dt.float8e4
    DR = mybir.MatmulPerfMode.DoubleRow
    AF = mybir.ActivationFunctionType

    nc = bacc.Bacc("TRN2", target_bir_lowering=False)

    x_d = nc.dram_tensor("x", [bpc, IN_C, HW], bf16, kind="ExternalInput")
    xt_d = nc.dram_tensor("xt8", [bpc, 128, HW], fp8, kind="ExternalInput")
    m_d = nc.dram_tensor("m", [IN_C, IN_C], bf16, kind="ExternalInput")
    h_d = nc.dram_tensor("hcol", [IN_C, 1], bf16, kind="ExternalInput")
    ident_d = nc.dram_tensor("ident16", [128, 128], bf16,
                             kind="ExternalInput")
    wvT_d = nc.dram_tensor("wvT", [IN_C, HIDE_C], bf16, kind="ExternalInput")
    woT_d = nc.dram_tensor("woT8", [128, HIDE_C], fp8, kind="ExternalInput")
    # packed per-partition fp32 bias columns: [Bv_lo, Bv_hi, Bo]
    bias_d = nc.dram_tensor("biases", [128, 3], f32, kind="ExternalInput")
    out_d = nc.dram_tensor("out", [bpc, OUT_C, HW], f32, kind="ExternalOutput")

    # supergroups of 4 positions (last: 1)
    sgroups = [(p, 4) for p in range(0, NP - 1, 4)] + [(NP - 1, 1)]
    # x DMA chunks: position-aligned so every 2-pos read stays inside one
    xchunks = [(0, 8), (8, 8), (16, 8), (24, 8), (32, 8), (40, 9)]

    with tile.TileContext(nc) as tc:
        with (
            tc.tile_pool(name="singles", bufs=1) as singles,
            tc.tile_pool(name="xwin", bufs=3) as xwin_pool,
            tc.tile_pool(name="xt8p", bufs=2) as xt8_pool,
            tc.tile_pool(name="u_sb", bufs=4) as u_sb_pool,
            tc.tile_pool(name="xa_sb", bufs=4) as xa_sb_pool,
            tc.tile_pool(name="g_sb", bufs=3) as g_sb_pool,
            tc.tile_pool(name="attn_sb", bufs=2) as attn_pool,
            tc.tile_pool(name="small_sb", bufs=2) as small_pool,
            tc.tile_pool(name="y_sb", bufs=3) as y_pool,
            tc.tile_pool(name="ps_work", bufs=3, space="PSUM") as ps_work,
            tc.tile_pool(name="ps_dots", bufs=1, space="PSUM") as ps_dots,
            tc.tile_pool(name="ps_av", bufs=2, space="PSUM") as ps_av,
        ):
            # ---- weights / constants (resident) ----
            # DMA-completion waits are monotonic counter thresholds, so a
            # read of DMA #k implicitly waits all earlier DMAs too. Emit
            # m_sb (needed by the first u-conv) BEFORE the image-0 x load,
            # and everything else after it (see start_dmas(0) call order).
            m_sb = singles.tile([128, IN_C], bf16)
            nc.sync.dma_start(out=m_sb, in_=m_d.ap())

            def load_consts():
                nc.sync.dma_start(out=h_sb, in_=h_d.ap())
                nc.sync.dma_start(out=ident16, in_=ident_d.ap())
                nc.sync.dma_start(out=wvT, in_=wvT_d.ap())
                nc.sync.dma_start(out=woT8, in_=woT_d.ap())
                nc.sync.dma_start(out=biases, in_=bias_d.ap())

            h_sb = singles.tile([128, 1], bf16)
            ident16 = singles.tile([128, 128], bf16)
            wvT = singles.tile([128, HIDE_C], bf16)
            woT8 = singles.tile([128, HIDE_C], fp8)
            biases = singles.tile([128, 3], f32)
            bv_ap = [biases[:, 0:1], biases[:, 1:2]]
            bo_ap = biases[:, 2:3]

            # 0.125 so the softmax-sum reciprocal yields attn8 = 8*attn
            ones_mat = singles.tile([128, 128], bf16)
            nc.vector.memset(ones_mat, 0.125)
            ones_row = singles.tile([1, T], bf16)
            nc.vector.memset(ones_row, 1.0)
            sel4 = singles.tile([128, 1], bf16)
            nc.vector.memset(sel4, 0.0)
            for t4 in range(4):
                nc.vector.memset(sel4[32 * t4:32 * t4 + 1, :], 1.0)

            woT8_3 = woT8.rearrange("p (kc m) -> p kc m", kc=2)

            # per-image state carried between pipeline slots
            xw_t = [None] * bpc
            xt_t = [None] * bpc
            dots_tiles = [None] * bpc
            attn_state = [None] * bpc

            def start_dmas(img):
                x_winb = xwin_pool.tile([128, NP * T], bf16, tag="xwin",
                                        name=f"xw{img}")
                for p0, pc in xchunks:
                    nc.sync.dma_start(
                        out=x_winb[:, p0 * T:(p0 + pc) * T],
                        in_=x_d.ap()[img, :, p0 * T:(p0 + pc) * T])
                xt8_sb = xt8_pool.tile([128, NP * T], fp8, tag="xt8",
                                       name=f"xt{img}")
                nc.sync.dma_start(out=xt8_sb, in_=xt_d.ap()[img])
                xw_t[img] = x_winb
                xt_t[img] = xt8_sb

            def phase1_steps(img):
                """Yield small closures: u-convs + deferred dots, c-trick."""
                x_winb = xw_t[img]
                dots_t = ps_dots.tile([128, 512], f32, tag="dots",
                                      name=f"dots{img}")
                dots_tiles[img] = dots_t
                dots = [dots_t[:, 0:T], dots_t[:, T:2 * T]]
                chunk_starts = list(range(0, NP, 2))
                pend = []

                def u_conv(ci, p0):
                    npos = min(2, NP - p0)
                    N = npos * T
                    base = p0 * T
                    u_ps = ps_work.tile([128, 512], f32, tag="pwork",
                                        name=f"ups{img}_{ci}")
                    nc.tensor.matmul(u_ps[:, :N], lhsT=m_sb,
                                     rhs=x_winb[:, base:base + N],
                                     start=True, stop=True)
                    u_sbt = u_sb_pool.tile([128, 512], bf16, tag="u")
                    nc.vector.tensor_copy(u_sbt[:, :N], u_ps[:, :N])
                    return u_sbt

                def dots_mms(ci, p0, u_sbt):
                    npos = min(2, NP - p0)
                    base = p0 * T
                    first = ci == 0
                    for pi in range(npos):
                        for jh in (0, 1):
                            nc.tensor.matmul(
                                dots[jh],
                                lhsT=u_sbt[:, pi * T + jh * 128:
                                           pi * T + jh * 128 + 128],
                                rhs=x_winb[:, base + pi * T:
                                           base + (pi + 1) * T],
                                start=first and pi == 0 and jh == 0,
                                stop=False,
                                skip_group_check=True)

                def step(ci, p0):
                    u_sbt = u_conv(ci, p0)
                    if len(pend) >= 2:
                        dots_mms(*pend.pop(0))
                    pend.append((ci, p0, u_sbt))

                for ci, p0 in enumerate(chunk_starts):
                    yield (lambda ci=ci, p0=p0: step(ci, p0))

                def flush():
                    while pend:
                        dots_mms(*pend.pop(0))
                yield flush

                def c_strips(ps, pe):
                    nstrip = [13, 12, 12, 12]
                    for p in range(ps, pe):
                        t4 = p % 4
                        seen = p // 4 + 1
                        nc.tensor.matmul(
                            c_row_ps[32 * t4:32 * t4 + 1, 0:T],
                            lhsT=h_sb,
                            rhs=x_winb[:, p * T:(p + 1) * T],
                            start=seen == 1,
                            stop=seen == nstrip[t4],
                            tile_position=(0, 32 * t4),
                            skip_group_check=True)

                def c_part1():
                    nonlocal c_row_ps
                    c_row_big = ps_av.tile([128, 1024], f32, tag="av",
                                           name=f"cps{img}")
                    c_row_ps = c_row_big[:, 0:512]
                    if img == 0:
                        # later images: stale finite PSUM zeroed by sel4
                        nc.vector.memset(c_row_ps[:, 0:T], 0.0)
                    c_strips(0, 25)

                def c_part2():
                    c_strips(25, NP)
                    c_all = small_pool.tile([128, T], bf16, tag="c4sb")
                    nc.vector.tensor_copy(c_all, c_row_ps[:, 0:T])
                    c_ps2 = ps_work.tile([128, 512], f32, tag="pwork",
                                         name=f"cps2{img}")
                    nc.tensor.matmul(c_ps2[0:1, 0:T], lhsT=sel4, rhs=c_all,
                                     start=True, stop=True)
                    c_row = small_pool.tile([1, T], bf16, tag="csb")
                    nc.vector.tensor_copy(c_row, c_ps2[0:1, 0:T])
                    for jh in (0, 1):
                        nc.tensor.matmul(
                            dots[jh],
                            lhsT=c_row[:, jh * 128:jh * 128 + 128],
                            rhs=ones_row, start=False, stop=jh == 1,
                            skip_group_check=True)

                c_row_ps = None
                yield c_part1
                yield c_part2

            def softmax(img):
                dots_t = dots_tiles[img]
                attn_e = attn_pool.tile([128, 512], bf16, tag="attne",
                                        name=f"attne{img}")
                nc.scalar.activation(attn_e, dots_t, AF.Exp)
                s_ps = ps_work.tile([128, 512], f32, tag="pwork",
                                    name=f"ssum{img}")
                for jc in (0, 1):
                    nc.tensor.matmul(s_ps[:, 0:T], lhsT=ones_mat,
                                     rhs=attn_e[:, jc * T:(jc + 1) * T],
                                     start=jc == 0, stop=jc == 1)
                r_sb = small_pool.tile([128, T], f32, tag="rsb")
                nc.vector.reciprocal_approx_fast(r_sb, s_ps[:, 0:T])
                attn8 = attn_pool.tile([128, 512], fp8, tag="attn",
                                       name=f"attn{img}")
                for jc in (0, 1):
                    # SBUF-only op -> GPSIMD (it cannot read PSUM)
                    nc.gpsimd.tensor_mul(attn8[:, jc * T:(jc + 1) * T],
                                         attn_e[:, jc * T:(jc + 1) * T],
                                         r_sb)
                attn_state[img] = attn8.rearrange("p (jc t) -> p jc t", jc=2)

            def xa_mms(img, sg):
                """fp8 DR: xa = (8x)@(8attn), one MM per position."""
                p0, cnt = sg
                attn8_3 = attn_state[img]
                xt8_sb = xt_t[img]
                tiles = []
                for ph in range(0, cnt, 2):
                    n2 = min(2, cnt - ph) * T
                    xa_ps = ps_work.tile([128, 512], f32, tag="pwork",
                                         name=f"xaps{img}_{p0}_{ph}")
                    for pi in range(ph, min(ph + 2, cnt)):
                        p = p0 + pi
                        xt3 = xt8_sb[:, p * T:(p + 1) * T].rearrange(
                            "p (jc c) -> p jc c", jc=2)
                        nc.tensor.matmul(
                            xa_ps[:, (pi - ph) * T:(pi - ph + 1) * T],
                            lhsT=xt3, rhs=attn8_3,
                            start=True, stop=True,
                            perf_mode=DR, skip_group_check=True)
                    xa_bf = xa_sb_pool.tile([128, 512], bf16, tag="xa")
                    nc.vector.tensor_copy(xa_bf[:, :n2], xa_ps[:, :n2])
                    tiles.append(xa_bf)
                return tiles

            def av_stage(img, gi, xa_tiles, fns):
                """av (bf16, stationary wv strips) -> gelu1 (one ACT inst
                per kc over [128, cnt*T]); phase-1 fillers keep the PE fed
                while ACT runs. Returns g8 for the (one-SG-later) o-stage."""
                p0, cnt = sgroups[gi]
                av_ts = []
                for kc in (0, 1):
                    av_t = ps_av.tile([128, 1024], f32, tag="av",
                                      name=f"av{img}_{gi}_{kc}")
                    for ph in range(0, cnt, 2):
                        n2 = min(2, cnt - ph) * T
                        nc.tensor.matmul(
                            av_t[:, ph * T:ph * T + n2],
                            lhsT=wvT[:, kc * 128:(kc + 1) * 128],
                            rhs=xa_tiles[ph // 2][:, :n2],
                            start=True, stop=True,
                            skip_group_check=True)
                    av_ts.append(av_t)

                g8 = g_sb_pool.tile([128, 2048], fp8, tag="g8")
                N = cnt * T
                for kc in (0, 1):
                    nc.scalar.activation(
                        g8[:, kc * 1024:kc * 1024 + N], av_ts[kc][:, :N],
                        AF.Gelu, bias=bv_ap[kc], scale=1.0 / 64)

                # fill PE while ACT runs gelu1
                for fn in fns:
                    fn()
                return g8

            def o_stage(img, gi, g8):
                """Residual (16*I) + out-conv (fp8 DR) into one PSUM group,
                gelu2 (scale=1/16) and streamed store. Runs one SG behind
                its av-stage so o-DR never waits on gelu1."""
                p0, cnt = sgroups[gi]
                base = p0 * T
                x_winb = xw_t[img]
                g83 = g8.rearrange("p (kc n) -> p kc n", kc=2)
                for ph in range(0, cnt, 2):
                    n2 = min(2, cnt - ph) * T
                    o_ps = ps_work.tile([128, 512], f32, tag="pwork",
                                        name=f"ops{img}_{gi}_{ph}")
                    nc.tensor.matmul(
                        o_ps[:, :n2],
                        lhsT=ident16,
                        rhs=x_winb[:, base + ph * T:base + ph * T + n2],
                        start=True, stop=False,
                        skip_group_check=True)
                    nc.tensor.matmul(
                        o_ps[:, :n2],
                        lhsT=woT8_3,
                        rhs=g83[:, :, ph * T:ph * T + n2],
                        start=False, stop=True,
                        perf_mode=DR, skip_group_check=True)
                    y_t = y_pool.tile([128, 512], f32, tag="y")
                    nc.scalar.activation(y_t[:, :n2], o_ps[:, :n2],
                                         AF.Gelu, bias=bo_ap,
                                         scale=1.0 / 16)
                    nc.sync.dma_start(
                        out=out_d.ap()[img, :,
                                       base + ph * T:base + ph * T + n2],
                        in_=y_t[:, :n2])

            # ---- pipelined emission ----
            # Slot i runs phase2(i) while weaving in phase1(i+1) over its
            # first SGs; softmax(i+1) is emitted mid-slot right after the
            # weave finishes, so the exp -> ssum -> recip -> normalize
            # chain (plus the two ACT exp-table loads) overlaps the tail
            # SGs of phase2(i) instead of stalling the slot boundary. The
            # next image's first xa matmuls are pre-emitted at slot end.
            start_dmas(0)
            load_consts()
            for fn in phase1_steps(0):
                fn()
            nsg = len(sgroups)
            pending = None          # (img, gi, g8) for the trailing o-stage
            for img in range(bpc):
                if img + 1 < bpc:
                    start_dmas(img + 1)
                    p1_next = list(phase1_steps(img + 1))
                else:
                    p1_next = []
                # next-image u-convs cover the exp-table load and the
                # softmax chain; the previous image's last o-stage flushes
                # here where gelu1 has long completed
                pre = min(5, len(p1_next))
                for fn in p1_next[:pre]:
                    fn()
                if pending is not None:
                    o_stage(*pending)
                    pending = None
                softmax(img)
                rest = p1_next[pre:]
                share = [len(rest) * (k + 1) // nsg for k in range(nsg)]
                # SG0's fillers go BEFORE the xa matmuls in the PE FIFO:
                # both xa(SG0) and xa(SG1) wait on the softmax chain
                for fn in rest[:share[0]]:
                    fn()
                done = share[0]
                xa_tiles = xa_mms(img, sgroups[0])
                for gi in range(nsg):
                    nxt = (xa_mms(img, sgroups[gi + 1])
                           if gi + 1 < nsg else None)
                    fns = [] if gi == 0 else rest[done:share[gi]]
                    if gi > 0:
                        done = share[gi]
                    g8 = av_stage(img, gi, xa_tiles, fns)
                    if pending is not None:
                        o_stage(*pending)
                    pending = (img, gi, g8)
                    xa_tiles = nxt
            if pending is not None:
                o_stage(*pending)

    nc.compile()
    return nc


def fold_params(wq, gq, bq, mq, vq, wk, gk, bk, mk, vk,
                wv, gv, bv, mv, vv, wo, bo, go, bbo, mo, vo):
    """Host-side BN/bias folding.

    Returns (M, h, ident16, wvT, woT8, biases)."""
    import ml_dtypes
    bf16 = ml_dtypes.bfloat16
    e4m3 = ml_dtypes.float8_e4m3

    aq = gq / np.sqrt(vq + EPS)
    wq_f = (SCALE * aq)[:, None] * wq
    Bq = SCALE * (bq - aq * mq)

    ak = gk / np.sqrt(vk + EPS)
    wk_f = ak[:, None] * wk          # k bias drops (softmax shift invariance)

    M = wk_f.T @ wq_f                # dots_T = sum_p (M^T x_p)^T x_p
    hv = wk_f.T @ Bq                 # c[j] = sum_p hv . x_p[:, j]

    av = gv / np.sqrt(vv + EPS)
    wv_f = av[:, None] * wv
    Bv = bv - av * mv                # applied inside gelu1 (rows of attn
                                     # sum to 1, so Bv passes through av)

    ao = go / np.sqrt(vo + EPS)
    wo_f = 16.0 * ao[:, None] * wo   # x16; gelu2 unscales via scale=1/16
    Bo = ao * (bo - mo) + bbo        # conv bias + BN fold, in gelu2's bias

    # woT8[h, kc*128+oc] = wo_f[oc, kc*128+h]  (fp8, DoubleRow layout)
    woT8 = np.empty((128, 256), dtype=np.float32)
    for kc in range(2):
        woT8[:, kc * 128:(kc + 1) * 128] = wo_f[:, kc * 128:(kc + 1) * 128].T
    biases = np.stack([Bv[:128], Bv[128:], Bo], axis=1).astype(F32)
    return (np.ascontiguousarray(M).astype(bf16),
            np.ascontiguousarray(hv[:, None]).astype(bf16),
            (16.0 * np.eye(128, dtype=np.float32)).astype(bf16),
            np.ascontiguousarray(wv_f.T).astype(bf16),
            np.ascontiguousarray(woT8).astype(e4m3),
            biases)


_CACHED = {}


def _get_nc(bpc=BPC):
    if bpc not in _CACHED:
        _CACHED[bpc] = build_bass_kernel(bpc)
    return _CACHED[bpc]


def _to_win(x):
    """[n, c, H, W] image layout -> [n, c, p*T + j] win layout (host)."""
    n, c = x.shape[:2]
    x = x.reshape(n, c, H1, WS, W1, WS).transpose(0, 1, 3, 5, 2, 4)
    return np.ascontiguousarray(x.reshape(n, c, HW))


def _to_win_tok8(xw):
    """win layout [n, c, p*T + t] -> token-major fp8 8*x with layout
    [n, j, p*T + jc*128 + c]  (t = jc*128 + j)."""
    import ml_dtypes
    n = xw.shape[0]
    xv = xw.reshape(n, 128, NP, 2, 128)          # [n, c, p, jc, j]
    xt = xv.transpose(0, 4, 2, 3, 1)             # [n, j, p, jc, c]
    return np.ascontiguousarray(
        (8.0 * xt.astype(F32)).reshape(n, 128, HW)
    ).astype(ml_dtypes.float8_e4m3)


def _from_win(y):
    """[n, c, p*T + j] win layout -> [n, c, H, W] image layout (host)."""
    n, c = y.shape[:2]
    y = y.reshape(n, c, WS, WS, H1, W1).transpose(0, 1, 4, 2, 5, 3)
    return y.reshape(n, c, H, W)


def make_in_maps(inputs):
    x = np.asarray(inputs["x"], F32)
    m, hv, ident16, wvT, woT8, biases = fold_params(
        *[np.asarray(inputs[k], F32) for k in
          ("wq", "gq", "bq", "mq", "vq", "wk", "gk", "bk", "mk", "vk",
           "wv", "gv", "bv", "mv", "vv", "wo", "bo", "go", "bbo", "mo", "vo")]
    )
    import ml_dtypes
    in_maps = []
    for c in range(NCORES):
        xw = _to_win(x[c * BPC:(c + 1) * BPC])
        xs = xw.astype(ml_dtypes.bfloat16)
        xt8 = _to_win_tok8(xs.astype(F32))
        in_maps.append({"x": xs, "xt8": xt8, "m": m, "hcol": hv,
                        "ident16": ident16, "wvT": wvT, "woT8": woT8,
                        "biases": biases})
    return in_maps


def kernel(**inputs):
    from concourse.bass_utils import run_bass_kernel_spmd

    in_maps = make_in_maps(inputs)
    nc = _get_nc(BPC)
    res = run_bass_kernel_spmd(nc, in_maps, list(range(NCORES)))
    outs = [_from_win(res.results[c]["out"].reshape(BPC, OUT_C, HW))
            for c in range(NCORES)]
    return np.concatenate(outs, axis=0)


# revision 28
# speedup vs baseline: 1.0115x; 1.0115x over previous
"""
Trainium2 Bass kernel for nn_Attention_335007449901 (sparse window attention).

Model (per image, eval mode):
  q = BN(conv1x1(x, wq)); k = BN(conv1x1(x, wk)); v = BN(conv1x1(x, wv))
  7x7 windows over the 112x112 image -> T=256 window tokens, token
  features = (channel, within-window position p) pairs.
  dots[i,j] = <q_i, k_j> * 0.125 ; attn = softmax_j ; out = attn @ v
  y = gelu(out); z = BN(conv1x1(y, wo) + bo); out = gelu(z + x)

Sharding: pure data parallel over batch, 4 images per core on 8 cores.

Scheme (v3):
  * Window permute on the host both ways; device sees win layout
    [c, p*T + j] contiguous. BNs folded into weights on the host;
    q/k never materialize (dots_T via u = M^T x); k bias drops; the
    q-bias row c[j] via 4-up col-tiled M=1 matmuls.
  * Contraction reorder for the value path: av = wv @ (x @ attn)
    instead of (wv x) @ attn -- the attention-average contracts over
    the 128 input channels, not 256 hidden ones. xa = x @ attn runs
    as one fp8 DoubleRow matmul per position (contraction 256 = both
    token halves, stationary = a host-provided token-major fp8 copy
    of x), then av = wv @ xa in bf16 with a constant stationary wv.
    This kills the per-position v-conv casts entirely.
  * The out-conv runs in fp8 DoubleRow (contraction 256 = both hidden
    halves) with stationary 16*wo. The residual rides the same PSUM
    group as a 16*I identity matmul (emitted first, start=True), so
    gelu2 reads PSUM directly with scale=1/16.
  * Scales keep fp8 in range: xT8 = 8*x, attn8 = 8*attn (via 0.125
    ones in the softmax-sum matmul), wo8 = 16*wo; the 1/64 and 1/16
    fold into the gelus' input scale; Bv/Bo ride the gelus'
    per-partition bias (Bv passes through because attn rows sum to 1).
  * Phase 2 works in supergroups of 4 positions: av PSUM is one
    [128, 1024] 2-bank tile per (SG, kc) so gelu1 is a single ACT
    instruction with a uniform per-partition bias.
  * Cross-image software pipeline: phase 1 of image i+1 (u-convs,
    dots, c-trick -- PE/DVE heavy, ACT idle) is emitted interleaved
    into phase 2 of image i (ACT heavy), two steps per supergroup,
    so no engine sits idle at image boundaries. xa matmuls run one
    SG ahead of their av consumers to hide the DVE cast latency.
  * Engine split: PE matmuls; ACT exp + both gelus; DVE all
    PSUM->SBUF casts + reciprocal; GPSIMD the softmax normalize muls
    (it cannot read PSUM).
"""

import numpy as np

IN_C = 128
HIDE_C = 256
HC2 = 128
OUT_C = 128
WS = 7
SCALE = 0.125
EPS = 1e-5
B, H, W = 32, 112, 112
HW = H * W          # 12544
H1 = H // WS        # 16
W1 = W // WS        # 16
T = H1 * W1         # 256 windows
NP = WS * WS        # 49 positions
NCORES = 8
BPC = B // NCORES   # images per core

F32 = np.float32


def build_bass_kernel(bpc=BPC):
    import concourse.bass as bass
    import concourse.tile as tile
    import concourse.mybir as mybir
    from concourse import bacc

    f32 = mybir.dt.float32
    bf16 = mybir.dt.bfloat16
    fp8 = mybir.dt.float8e4
    DR = mybir.MatmulPerfMode.DoubleRow
    AF = mybir.ActivationFunctionType

    nc = bacc.Bacc("TRN2", target_bir_lowering=False)

    x_d = nc.dram_tensor("x", [bpc, IN_C, HW], bf16, kind="ExternalInput")
    xt_d = nc.dram_tensor("xt8", [bpc, 128, HW], fp8, kind="ExternalInput")
    m_d = nc.dram_tensor("m", [IN_C, IN_C], bf16, kind="ExternalInput")
    h_d = nc.dram_tensor("hcol", [IN_C, 1], bf16, kind="ExternalInput")
    ident_d = nc.dram_tensor("ident16", [128, 128], bf16,
                             kind="ExternalInput")
    wvT_d = nc.dram_tensor("wvT", [IN_C, HIDE_C], bf16, kind="ExternalInput")
    woT_d = nc.dram_tensor("woT8", [128, HIDE_C], fp8, kind="ExternalInput")
    # packed per-partition fp32 bias columns: [Bv_lo, Bv_hi, Bo]
    bias_d = nc.dram_tensor("biases", [128, 3], f32, kind="ExternalInput")
    out_d = nc.dram_tensor("out", [bpc, OUT_C, HW], f32, kind="ExternalOutput")

    # supergroups of 4 positions (last: 1)
    sgroups = [(p, 4) for p in range(0, NP - 1, 4)] + [(NP - 1, 1)]
    # x DMA chunks: position-aligned so every 2-pos read stays inside one
    xchunks = [(0, 8), (8, 8), (16, 8), (24, 8), (32, 8), (40, 9)]

    with tile.TileContext(nc) as tc:
        with (
            tc.tile_pool(name="singles", bufs=1) as singles,
            tc.tile_pool(name="xwin", bufs=3) as xwin_pool,
            tc.tile_pool(name="xt8p", bufs=2) as xt8_pool,
            tc.tile_pool(name="u_sb", bufs=4) as u_sb_pool,
            tc.tile_pool(name="xa_sb", bufs=4) as xa_sb_pool,
            tc.tile_pool(name="g_sb", bufs=3) as g_sb_pool,
            tc.tile_pool(name="attn_sb", bufs=2) as attn_pool,
            tc.tile_pool(name="small_sb", bufs=2) as small_pool,
            tc.tile_pool(name="y_sb", bufs=3) as y_pool,
            tc.tile_pool(name="ps_work", bufs=3, space="PSUM") as ps_work,
            tc.tile_pool(name="ps_dots", bufs=1, space="PSUM") as ps_dots,
            tc.tile_pool(name="ps_av", bufs=2, space="PSUM") as ps_av,
        ):
            # ---- weights / constants (resident) ----
            # DMA-completion waits are monotonic counter thresholds, so a
            # read of DMA #k implicitly waits all earlier DMAs too. Emit
            # m_sb (needed by the first u-conv) BEFORE the image-0 x load,
            # and everything else after it (see start_dmas(0) call order).
            m_sb = singles.tile([128, IN_C], bf16)
            nc.sync.dma_start(out=m_sb, in_=m_d.ap())

            def load_consts():
                nc.sync.dma_start(out=h_sb, in_=h_d.ap())
                nc.sync.dma_start(out=ident16, in_=ident_d.ap())
                nc.sync.dma_start(out=wvT, in_=wvT_d.ap())
                nc.sync.dma_start(out=woT8, in_=woT_d.ap())
                nc.sync.dma_start(out=biases, in_=bias_d.ap())

            h_sb = singles.tile([128, 1], bf16)
            ident16 = singles.tile([128, 128], bf16)
            wvT = singles.tile([128, HIDE_C], bf16)
            woT8 = singles.tile([128, HIDE_C], fp8)
            biases = singles.tile([128, 3], f32)
            bv_ap = [biases[:, 0:1], biases[:, 1:2]]
            bo_ap = biases[:, 2:3]

            # 0.125 so the softmax-sum reciprocal yields attn8 = 8*attn
            ones_mat = singles.tile([128, 128], bf16)
            nc.vector.memset(ones_mat, 0.125)
            ones_row = singles.tile([1, T], bf16)
            nc.vector.memset(ones_row, 1.0)
            sel4 = singles.tile([128, 1], bf16)
            nc.vector.memset(sel4, 0.0)
            for t4 in range(4):
                nc.vector.memset(sel4[32 * t4:32 * t4 + 1, :], 1.0)

            woT8_3 = woT8.rearrange("p (kc m) -> p kc m", kc=2)

            # per-image state carried between pipeline slots
            xw_t = [None] * bpc
            xt_t = [None] * bpc
            dots_tiles = [None] * bpc
            attn_state = [None] * bpc

            def start_dmas(img):
                x_winb = xwin_pool.tile([128, NP * T], bf16, tag="xwin",
                                        name=f"xw{img}")
                for p0, pc in xchunks:
                    nc.sync.dma_start(
                        out=x_winb[:, p0 * T:(p0 + pc) * T],
                        in_=x_d.ap()[img, :, p0 * T:(p0 + pc) * T])
                xt8_sb = xt8_pool.tile([128, NP * T], fp8, tag="xt8",
                                       name=f"xt{img}")
                nc.sync.dma_start(out=xt8_sb, in_=xt_d.ap()[img])
                xw_t[img] = x_winb
                xt_t[img] = xt8_sb

            def phase1_steps(img):
                """Yield small closures: u-convs + deferred dots, c-trick."""
                x_winb = xw_t[img]
                dots_t = ps_dots.tile([128, 512], f32, tag="dots",
                                      name=f"dots{img}")
                dots_tiles[img] = dots_t
                dots = [dots_t[:, 0:T], dots_t[:, T:2 * T]]
                chunk_starts = list(range(0, NP, 2))
                pend = []

                def u_conv(ci, p0):
                    npos = min(2, NP - p0)
                    N = npos * T
                    base = p0 * T
                    u_ps = ps_work.tile([128, 512], f32, tag="pwork",
                                        name=f"ups{img}_{ci}")
                    nc.tensor.matmul(u_ps[:, :N], lhsT=m_sb,
                                     rhs=x_winb[:, base:base + N],
                                     start=True, stop=True)
                    u_sbt = u_sb_pool.tile([128, 512], bf16, tag="u")
                    nc.vector.tensor_copy(u_sbt[:, :N], u_ps[:, :N])
                    return u_sbt

                def dots_mms(ci, p0, u_sbt):
                    npos = min(2, NP - p0)
                    base = p0 * T
                    first = ci == 0
                    for pi in range(npos):
                        for jh in (0, 1):
                            nc.tensor.matmul(
                                dots[jh],
                                lhsT=u_sbt[:, pi * T + jh * 128:
                                           pi * T + jh * 128 + 128],
                                rhs=x_winb[:, base + pi * T:
                                           base + (pi + 1) * T],
                                start=first and pi == 0 and jh == 0,
                                stop=False,
                                skip_group_check=True)

                def step(ci, p0):
                    u_sbt = u_conv(ci, p0)
                    if len(pend) >= 2:
                        dots_mms(*pend.pop(0))
                    pend.append((ci, p0, u_sbt))

                for ci, p0 in enumerate(chunk_starts):
                    yield (lambda ci=ci, p0=p0: step(ci, p0))

                def flush():
                    while pend:
                        dots_mms(*pend.pop(0))
                yield flush

                def c_strips(ps, pe):
                    nstrip = [13, 12, 12, 12]
                    for p in range(ps, pe):
                        t4 = p % 4
                        seen = p // 4 + 1
                        nc.tensor.matmul(
                            c_row_ps[32 * t4:32 * t4 + 1, 0:T],
                            lhsT=h_sb,
                            rhs=x_winb[:, p * T:(p + 1) * T],
                            start=seen == 1,
                            stop=seen == nstrip[t4],
                            tile_position=(0, 32 * t4),
                            skip_group_check=True)

                def c_part1():
                    nonlocal c_row_ps
                    c_row_big = ps_av.tile([128, 1024], f32, tag="av",
                                           name=f"cps{img}")
                    c_row_ps = c_row_big[:, 0:512]
                    if img == 0:
                        # later images: stale finite PSUM zeroed by sel4
                        nc.vector.memset(c_row_ps[:, 0:T], 0.0)
                    c_strips(0, 25)

                def c_part2():
                    c_strips(25, NP)
                    c_all = small_pool.tile([128, T], bf16, tag="c4sb")
                    nc.vector.tensor_copy(c_all, c_row_ps[:, 0:T])
                    c_ps2 = ps_work.tile([128, 512], f32, tag="pwork",
                                         name=f"cps2{img}")
                    nc.tensor.matmul(c_ps2[0:1, 0:T], lhsT=sel4, rhs=c_all,
                                     start=True, stop=True)
                    c_row = small_pool.tile([1, T], bf16, tag="csb")
                    nc.vector.tensor_copy(c_row, c_ps2[0:1, 0:T])
                    for jh in (0, 1):
                        nc.tensor.matmul(
                            dots[jh],
                            lhsT=c_row[:, jh * 128:jh * 128 + 128],
                            rhs=ones_row, start=False, stop=jh == 1,
                            skip_group_check=True)

                c_row_ps = None
                yield c_part1
                yield c_part2

            def softmax(img):
                dots_t = dots_tiles[img]
                attn_e = attn_pool.tile([128, 512], bf16, tag="attne",
                                        name=f"attne{img}")
                nc.scalar.activation(attn_e, dots_t, AF.Exp)
                s_ps = ps_work.tile([128, 512], f32, tag="pwork",
                                    name=f"ssum{img}")
                for jc in (0, 1):
                    nc.tensor.matmul(s_ps[:, 0:T], lhsT=ones_mat,
                                     rhs=attn_e[:, jc * T:(jc + 1) * T],
                                     start=jc == 0, stop=jc == 1)
                r_sb = small_pool.tile([128, T], f32, tag="rsb")
                nc.vector.reciprocal_approx_fast(r_sb, s_ps[:, 0:T])
                attn8 = attn_pool.tile([128, 512], fp8, tag="attn",
                                       name=f"attn{img}")
                for jc in (0, 1):
                    # SBUF-only op -> GPSIMD (it cannot read PSUM)
                    nc.gpsimd.tensor_mul(attn8[:, jc * T:(jc + 1) * T],
                                         attn_e[:, jc * T:(jc + 1) * T],
                                         r_sb)
                attn_state[img] = attn8.rearrange("p (jc t) -> p jc t", jc=2)

            def xa_mms(img, sg):
                """fp8 DR: xa = (8x)@(8attn), one MM per position."""
                p0, cnt = sg
                attn8_3 = attn_state[img]
                xt8_sb = xt_t[img]
                tiles = []
                for ph in range(0, cnt, 2):
                    n2 = min(2, cnt - ph) * T
                    xa_ps = ps_work.tile([128, 512], f32, tag="pwork",
                                         name=f"xaps{img}_{p0}_{ph}")
                    for pi in range(ph, min(ph + 2, cnt)):
                        p = p0 + pi
                        xt3 = xt8_sb[:, p * T:(p + 1) * T].rearrange(
                            "p (jc c) -> p jc c", jc=2)
                        nc.tensor.matmul(
                            xa_ps[:, (pi - ph) * T:(pi - ph + 1) * T],
                            lhsT=xt3, rhs=attn8_3,
                            start=True, stop=True,
                            perf_mode=DR, skip_group_check=True)
                    xa_bf = xa_sb_pool.tile([128, 512], bf16, tag="xa")
                    nc.vector.tensor_copy(xa_bf[:, :n2], xa_ps[:, :n2])
                    tiles.append(xa_bf)
                return tiles

            def av_stage(img, gi, xa_tiles, fns):
                """av (bf16, stationary wv strips) -> gelu1 (one ACT inst
                per kc over [128, cnt*T]); phase-1 fillers keep the PE fed
                while ACT runs. Returns g8 for the (one-SG-later) o-stage."""
                p0, cnt = sgroups[gi]
                av_ts = []
                for kc in (0, 1):
                    av_t = ps_av.tile([128, 1024], f32, tag="av",
                                      name=f"av{img}_{gi}_{kc}")
                    for ph in range(0, cnt, 2):
                        n2 = min(2, cnt - ph) * T
                        nc.tensor.matmul(
                            av_t[:, ph * T:ph * T + n2],
                            lhsT=wvT[:, kc * 128:(kc + 1) * 128],
                            rhs=xa_tiles[ph // 2][:, :n2],
                            start=True, stop=True,
                            skip_group_check=True)
                    av_ts.append(av_t)

                g8 = g_sb_pool.tile([128, 2048], fp8, tag="g8")
                N = cnt * T
                for kc in (0, 1):
                    nc.scalar.activation(
                        g8[:, kc * 1024:kc * 1024 + N], av_ts[kc][:, :N],
                        AF.Gelu, bias=bv_ap[kc], scale=1.0 / 64)

                # fill PE while ACT runs gelu1
                for fn in fns:
                    fn()
                return g8

            def o_stage(img, gi, g8):
                """Residual (16*I) + out-conv (fp8 DR) into one PSUM group,
                gelu2 (scale=1/16) and streamed store. Runs one SG behind
                its av-stage so o-DR never waits on gelu1."""
                p0, cnt = sgroups[gi]
                base = p0 * T
                x_winb = xw_t[img]
                g83 = g8.rearrange("p (kc n) -> p kc n", kc=2)
                for ph in range(0, cnt, 2):
                    n2 = min(2, cnt - ph) * T
                    o_ps = ps_work.tile([128, 512], f32, tag="pwork",
                                        name=f"ops{img}_{gi}_{ph}")
                    nc.tensor.matmul(
                        o_ps[:, :n2],
                        lhsT=ident16,
                        rhs=x_winb[:, base + ph * T:base + ph * T + n2],
                        start=True, stop=False,
                        skip_group_check=True)
                    nc.tensor.matmul(
                        o_ps[:, :n2],
                        lhsT=woT8_3,
                        rhs=g83[:, :, ph * T:ph * T + n2],
                        start=False, stop=True,
                        perf_mode=DR, skip_group_check=True)
                    y_t = y_pool.tile([128, 512], f32, tag="y")
                    nc.scalar.activation(y_t[:, :n2], o_ps[:, :n2],
                                         AF.Gelu, bias=bo_ap,
                                         scale=1.0 / 16)
                    nc.sync.dma_start(
                        out=out_d.ap()[img, :,
                                       base + ph * T:base + ph * T + n2],
                        in_=y_t[:, :n2])

            # ---- pipelined emission ----
            # Slot i runs phase2(i) while weaving in phase1(i+1) over its
            # first SGs; softmax(i+1) is emitted mid-slot right after the
            # weave finishes, so the exp -> ssum -> recip -> normalize
            # chain (plus the two ACT exp-table loads) overlaps the tail
            # SGs of phase2(i) instead of stalling the slot boundary. The
            # next image's first xa matmuls are pre-emitted at slot end.
            start_dmas(0)
            load_consts()
            for fn in phase1_steps(0):
                fn()
            nsg = len(sgroups)
            pending = None          # (img, gi, g8) for the trailing o-stage
            for img in range(bpc):
                if img + 1 < bpc:
                    start_dmas(img + 1)
                    p1_next = list(phase1_steps(img + 1))
                else:
                    p1_next = []
                # next-image u-convs cover the exp-table load and the
                # softmax chain; the previous image's last o-stage flushes
                # here where gelu1 has long completed
                pre = min(3, len(p1_next))
                for fn in p1_next[:pre]:
                    fn()
                if pending is not None:
                    o_stage(*pending)
                    pending = None
                softmax(img)
                rest = p1_next[pre:]
                share = [len(rest) * (k + 1) // nsg for k in range(nsg)]
                # SG0's fillers go BEFORE the xa matmuls in the PE FIFO:
                # both xa(SG0) and xa(SG1) wait on the softmax chain
                for fn in rest[:share[0]]:
                    fn()
                done = share[0]
                xa_tiles = xa_mms(img, sgroups[0])
                for gi in range(nsg):
                    nxt = (xa_mms(img, sgroups[gi + 1])
                           if gi + 1 < nsg else None)
                    fns = [] if gi == 0 else rest[done:share[gi]]
                    if gi > 0:
                        done = share[gi]
                    g8 = av_stage(img, gi, xa_tiles, fns)
                    if pending is not None:
                        o_stage(*pending)
                    pending = (img, gi, g8)
                    xa_tiles = nxt
            if pending is not None:
                o_stage(*pending)

    nc.compile()
    return nc


def fold_params(wq, gq, bq, mq, vq, wk, gk, bk, mk, vk,
                wv, gv, bv, mv, vv, wo, bo, go, bbo, mo, vo):
    """Host-side BN/bias folding.

    Returns (M, h, ident16, wvT, woT8, biases)."""
    import ml_dtypes
    bf16 = ml_dtypes.bfloat16
    e4m3 = ml_dtypes.float8_e4m3

    aq = gq / np.sqrt(vq + EPS)
    wq_f = (SCALE * aq)[:, None] * wq
    Bq = SCALE * (bq - aq * mq)

    ak = gk / np.sqrt(vk + EPS)
    wk_f = ak[:, None] * wk          # k bias drops (softmax shift invariance)

    M = wk_f.T @ wq_f                # dots_T = sum_p (M^T x_p)^T x_p
    hv = wk_f.T @ Bq                 # c[j] = sum_p hv . x_p[:, j]

    av = gv / np.sqrt(vv + EPS)
    wv_f = av[:, None] * wv
    Bv = bv - av * mv                # applied inside gelu1 (rows of attn
                                     # sum to 1, so Bv passes through av)

    ao = go / np.sqrt(vo + EPS)
    wo_f = 16.0 * ao[:, None] * wo   # x16; gelu2 unscales via scale=1/16
    Bo = ao * (bo - mo) + bbo        # conv bias + BN fold, in gelu2's bias

    # woT8[h, kc*128+oc] = wo_f[oc, kc*128+h]  (fp8, DoubleRow layout)
    woT8 = np.empty((128, 256), dtype=np.float32)
    for kc in range(2):
        woT8[:, kc * 128:(kc + 1) * 128] = wo_f[:, kc * 128:(kc + 1) * 128].T
    biases = np.stack([Bv[:128], Bv[128:], Bo], axis=1).astype(F32)
    return (np.ascontiguousarray(M).astype(bf16),
            np.ascontiguousarray(hv[:, None]).astype(bf16),
            (16.0 * np.eye(128, dtype=np.float32)).astype(bf16),
            np.ascontiguousarray(wv_f.T).astype(bf16),
            np.ascontiguousarray(woT8).astype(e4m3),
            biases)


_CACHED = {}


def _get_nc(bpc=BPC):
    if bpc not in _CACHED:
        _CACHED[bpc] = build_bass_kernel(bpc)
    return _CACHED[bpc]


def _to_win(x):
    """[n, c, H, W] image layout -> [n, c, p*T + j] win layout (host)."""
    n, c = x.shape[:2]
    x = x.reshape(n, c, H1, WS, W1, WS).transpose(0, 1, 3, 5, 2, 4)
    return np.ascontiguousarray(x.reshape(n, c, HW))


def _to_win_tok8(xw):
    """win layout [n, c, p*T + t] -> token-major fp8 8*x with layout
    [n, j, p*T + jc*128 + c]  (t = jc*128 + j)."""
    import ml_dtypes
    n = xw.shape[0]
    xv = xw.reshape(n, 128, NP, 2, 128)          # [n, c, p, jc, j]
    xt = xv.transpose(0, 4, 2, 3, 1)             # [n, j, p, jc, c]
    return np.ascontiguousarray(
        (8.0 * xt.astype(F32)).reshape(n, 128, HW)
    ).astype(ml_dtypes.float8_e4m3)


def _from_win(y):
    """[n, c, p*T + j] win layout -> [n, c, H, W] image layout (host)."""
    n, c = y.shape[:2]
    y = y.reshape(n, c, WS, WS, H1, W1).transpose(0, 1, 4, 2, 5, 3)
    return y.reshape(n, c, H, W)


def make_in_maps(inputs):
    x = np.asarray(inputs["x"], F32)
    m, hv, ident16, wvT, woT8, biases = fold_params(
        *[np.asarray(inputs[k], F32) for k in
          ("wq", "gq", "bq", "mq", "vq", "wk", "gk", "bk", "mk", "vk",
           "wv", "gv", "bv", "mv", "vv", "wo", "bo", "go", "bbo", "mo", "vo")]
    )
    import ml_dtypes
    in_maps = []
    for c in range(NCORES):
        xw = _to_win(x[c * BPC:(c + 1) * BPC])
        xs = xw.astype(ml_dtypes.bfloat16)
        xt8 = _to_win_tok8(xs.astype(F32))
        in_maps.append({"x": xs, "xt8": xt8, "m": m, "hcol": hv,
                        "ident16": ident16, "wvT": wvT, "woT8": woT8,
                        "biases": biases})
    return in_maps


def kernel(**inputs):
    from concourse.bass_utils import run_bass_kernel_spmd

    in_maps = make_in_maps(inputs)
    nc = _get_nc(BPC)
    res = run_bass_kernel_spmd(nc, in_maps, list(range(NCORES)))
    outs = [_from_win(res.results[c]["out"].reshape(BPC, OUT_C, HW))
            for c in range(NCORES)]
    return np.concatenate(outs, axis=0)
